# revision 1
# baseline (speedup 1.0000x reference)
"""Trainium2 Bass kernel for a GPT-style transformer block (no attn out-proj).

Sharding (8 cores): attention is tensor-parallel over heads -- core c handles
batch c//4 and heads [4*(c%4), 4*(c%4)+4) over the full 2048-token causal
sequence (no communication). The MLP is token-parallel (core c takes tokens
[512*(c%4), 512*(c%4)+512) of its batch). The attention outputs are exchanged
with one AllGather inside each 4-core batch group, and each core picks its
token columns with an indirect DMA driven by a per-core index input, keeping
the single SPMD program fully static.

All activations are kept feature-major ([C, T]) so no on-device transposes are
needed anywhere. Matmuls run in float32r (full PE rate, ~1.5e-4 relative error
per 128-wide dot, measured on HW). LayerNorm affine parameters and biases are
folded into the weight matrices on the host (exact rewrite).
"""

import numpy as np

B, T, C = 2, 2048, 1024
H, D = 16, 64
HPC = 4          # heads per core
G = 4            # cores per batch group
TCH = T // G     # tokens per core for the MLP (512)
N_CORES = 8
EPS = 1e-5
FC = 4 * C

_CACHE = {}
LAST_EXEC_NS = None
LAST_RESULTS = None


def _build():
    import concourse.tile as tile
    from concourse import bacc, mybir

    F32 = mybir.dt.float32
    F32R = mybir.dt.float32r
    I32 = mybir.dt.int32
    AF = mybir.ActivationFunctionType

    nc = bacc.Bacc("TRN2", target_bir_lowering=False, debug=False,
                   num_devices=N_CORES)

    def inp(name, shape, dt=F32R):
        return nc.dram_tensor(name, shape, dt, kind="ExternalInput").ap()

    x_t = inp("x_t", [C, T])
    x_res_t = inp("x_res_t", [C, TCH])
    w_qk = inp("w_qk", [C, 2 * HPC * D])          # [1024, 512]
    b_qk = inp("b_qk", [2 * HPC * D, 1], F32)
    w_v = inp("w_v", [C, HPC * D])                # [1024, 256]
    b_v = inp("b_v", [1, HPC * D])
    masks = inp("masks", [4, 128, 512])
    ones_col = inp("ones_col", [128, 1])
    ones_row = inp("ones_row", [1, 128])
    w_fc = inp("w_fc", [C, FC])
    b_fc = inp("b_fc", [FC, 1], F32)
    w_proj = inp("w_proj", [FC, C])
    b_proj = inp("b_proj", [C, 1], F32)
    g_idx = inp("g_idx", [C, 1], I32)
    v_ones = inp("v_ones", [128, T // 128, HPC, 1])

    out_t = nc.dram_tensor("out_t", [C, TCH], F32, kind="ExternalOutput").ap()

    CO = C // 128        # 8  C tiles
    FO = FC // 128       # 32 fc tiles
    NT = T // 512        # 4 token chunks
    TT = T // 128        # 16 token tiles

    with tile.TileContext(nc) as tc:
        # ---- persistent pools ----------------------------------------
        const_cm = tc.tile_pool(name="const", bufs=1)
        dram_cm = tc.tile_pool(name="dram", bufs=1, space="DRAM")
        qkv_cm = tc.tile_pool(name="qkv", bufs=1)
        const = const_cm.__enter__()
        dram = dram_cm.__enter__()
        qkv_pool = qkv_cm.__enter__()

        ones_c = const.tile([128, 1], F32R)
        ones_r = const.tile([1, 128], F32R)
        mask_sb = const.tile([128, 4, 512], F32R)
        nc.sync.dma_start(ones_c[:], ones_col)
        nc.sync.dma_start(ones_r[:], ones_row)
        nc.sync.dma_start(mask_sb[:], masks.rearrange("m p f -> p m f"))

        q_sb = qkv_pool.tile([128, 2, T], F32R)
        k_sb = qkv_pool.tile([128, 2, T], F32R)
        v_sb = qkv_pool.tile([128, TT, HPC * (D + 1)], F32R)   # [.,16,260]
        v4 = v_sb[:].rearrange("p t (h e) -> p t h e", h=HPC)
        nc.sync.dma_start(v4[:, :, :, D:D + 1], v_ones)

        cc_in = dram.tile([HPC * D, T], F32)                   # [256, 2048]
        cc_out = dram.tile([G * HPC * D, T], F32)              # [1024, 2048]

        # ---- phase 1: load x, LN1 (in place), QKV --------------------
        with tc.tile_pool(name="xh", bufs=1) as xh_pool, \
             tc.tile_pool(name="ln_ps", bufs=1, space="PSUM") as ln_ps, \
             tc.tile_pool(name="ln_bc_ps", bufs=2, space="PSUM") as ln_bc_ps, \
             tc.tile_pool(name="ln_sb", bufs=1) as ln_sb, \
             tc.tile_pool(name="row", bufs=1) as row_pool, \
             tc.tile_pool(name="sq", bufs=3) as sq_pool, \
             tc.tile_pool(name="wq", bufs=1) as wq_pool, \
             tc.tile_pool(name="qk_ps", bufs=3, space="PSUM") as qk_ps:

            xh = xh_pool.tile([128, CO, T], F32R)
            nc.sync.dma_start(xh[:], x_t.rearrange("(o p) t -> p o t", p=128))

            rstd_bc = ln_sb.tile([128, T], F32R)
            nm_bc = ln_sb.tile([128, T], F32R)
            inv_c = 1.0 / C
            for cn in range(NT):
                sl = slice(cn * 512, cn * 512 + 512)
                ps_s = ln_ps.tile([1, 512], F32, tag="ps_s")
                ps_q = ln_ps.tile([1, 512], F32, tag="ps_q")
                for o in range(CO):
                    sq = sq_pool.tile([128, 512], F32R)
                    nc.vector.tensor_mul(sq[:], xh[:, o, sl], xh[:, o, sl])
                    nc.tensor.matmul(ps_s[:], ones_c[:], xh[:, o, sl],
                                     start=(o == 0), stop=(o == CO - 1))
                    nc.tensor.matmul(ps_q[:], ones_c[:], sq[:],
                                     start=(o == 0), stop=(o == CO - 1))
                mu = row_pool.tile([1, 512], F32, tag="mu")
                var = row_pool.tile([1, 512], F32, tag="var")
                std = row_pool.tile([1, 512], F32, tag="std")
                rstd = row_pool.tile([1, 512], F32R, tag="rstd")
                nm = row_pool.tile([1, 512], F32R, tag="nm")
                nc.vector.tensor_scalar_mul(mu[:], ps_s[:], inv_c)
                nc.vector.tensor_mul(var[:], mu[:], mu[:])
                nc.vector.tensor_scalar_mul(std[:], ps_q[:], inv_c)
                nc.vector.tensor_tensor(var[:], std[:], var[:],
                                        mybir.AluOpType.subtract)
                nc.vector.tensor_scalar_add(var[:], var[:], EPS)
                nc.scalar.activation(std[:], var[:], AF.Sqrt)
                with nc.allow_low_precision(reason="f32r is 4-byte fp32"):
                    nc.vector.reciprocal(rstd[:], std[:])
                nc.vector.tensor_mul(nm[:], mu[:], rstd[:])
                nc.vector.tensor_scalar_mul(nm[:], nm[:], -1.0)

                pb = ln_bc_ps.tile([128, 512], F32, tag="bc")
                nc.tensor.matmul(pb[:], ones_r[:], rstd[:],
                                 start=True, stop=True)
                nc.scalar.copy(rstd_bc[:, sl], pb[:])
                pb2 = ln_bc_ps.tile([128, 512], F32, tag="bc")
                nc.tensor.matmul(pb2[:], ones_r[:], nm[:],
                                 start=True, stop=True)
                nc.scalar.copy(nm_bc[:, sl], pb2[:])

            for o in range(CO):
                nc.vector.tensor_mul(xh[:, o, :], xh[:, o, :], rstd_bc[:])
                nc.vector.tensor_add(xh[:, o, :], xh[:, o, :], nm_bc[:])

            # --- q, k ---
            wqk_sb = wq_pool.tile([128, CO, 512], F32R)
            bqk_sb = wq_pool.tile([128, 4], F32)
            nc.sync.dma_start(wqk_sb[:], w_qk.rearrange("(o p) m -> p o m", p=128))
            nc.sync.dma_start(bqk_sb[:], b_qk.rearrange("(o p) u -> p (o u)", p=128))
            for m in range(4):                       # 2 q tiles then 2 k tiles
                for cn in range(NT):
                    sl = slice(cn * 512, cn * 512 + 512)
                    pq = qk_ps.tile([128, 512], F32, tag="mmps")
                    for o in range(CO):
                        nc.tensor.matmul(pq[:], wqk_sb[:, o, m * 128:(m + 1) * 128],
                                         xh[:, o, sl],
                                         start=(o == 0), stop=(o == CO - 1))
                    dest = q_sb[:, m, sl] if m < 2 else k_sb[:, m - 2, sl]
                    nc.scalar.activation(dest, pq[:], AF.Identity,
                                         bias=bqk_sb[:, m:m + 1])

            # --- v (token-major) + ones column ---
            wv_sb = wq_pool.tile([128, CO, HPC * D], F32R)
            bv_sb = wq_pool.tile([1, HPC * D], F32R)
            nc.sync.dma_start(wv_sb[:], w_v.rearrange("(o p) m -> p o m", p=128))
            nc.sync.dma_start(bv_sb[:], b_v)
            for tt in range(TT):
                tsl = slice(tt * 128, tt * 128 + 128)
                pv_full = qk_ps.tile([128, 512], F32, tag="mmps", name="pv")
                pv = pv_full[:, 0:HPC * D]
                nc.tensor.matmul(pv[:], ones_r[:], bv_sb[:], start=True, stop=False)
                for o in range(CO):
                    nc.tensor.matmul(pv[:], xh[:, o, tsl], wv_sb[:, o, :],
                                     start=False, stop=(o == CO - 1))
                vview = v_sb[:, tt, :].rearrange("p (h e) -> p h e", e=D + 1)
                nc.vector.tensor_copy(
                    vview[:, :, 0:D], pv[:].rearrange("p (h e) -> p h e", e=D))

        # ---- phase 2: attention -------------------------------------
        with tc.tile_pool(name="a", bufs=2) as a_pool, \
             tc.tile_pool(name="s_ps", bufs=3, space="PSUM") as s_ps, \
             tc.tile_pool(name="y_ps", bufs=2, space="PSUM") as y_ps, \
             tc.tile_pool(name="nb_ps", bufs=2, space="PSUM") as nb_ps, \
             tc.tile_pool(name="ysm", bufs=3) as ysm:
            for h in range(HPC):
                po = 64 * (h % 2)
                pt = h // 2
                for qb in range(NT):
                    qsl = slice(qb * 512, qb * 512 + 512)
                    nkv = 4 * qb + 4
                    a_t = a_pool.tile([128, TT, 512], F32R, tag="a")
                    for t in range(nkv):
                        ps = s_ps.tile([128, 512], F32)
                        nc.tensor.matmul(
                            ps[:],
                            k_sb[po:po + 64, pt, t * 128:(t + 1) * 128],
                            q_sb[po:po + 64, pt, qsl],
                            start=True, stop=True)
                        nc.scalar.activation(a_t[:, t, :], ps[:], AF.Exp,
                                             scale=0.125)
                        if t >= 4 * qb:
                            nc.vector.tensor_mul(a_t[:, t, :], a_t[:, t, :],
                                                 mask_sb[:, t - 4 * qb, :])
                    py = y_ps.tile([D + 1, 512], F32)
                    for t in range(nkv):
                        nc.tensor.matmul(py[:],
                                         v_sb[:, t, h * (D + 1):(h + 1) * (D + 1)],
                                         a_t[:, t, :],
                                         start=(t == 0), stop=(t == nkv - 1))
                    rr = ysm.tile([1, 512], F32R, tag="rr")
                    with nc.allow_low_precision(reason="f32r is 4-byte fp32"):
                        nc.vector.reciprocal(rr[:], py[D:D + 1, :])
                    pb = nb_ps.tile([D, 512], F32)
                    nc.tensor.matmul(pb[:], ones_r[:, 0:D], rr[:],
                                     start=True, stop=True)
                    rb = ysm.tile([D, 512], F32R, tag="rb")
                    nc.scalar.copy(rb[:], pb[:])
                    y_hqb = ysm.tile([D, 512], F32, tag="y")
                    nc.vector.tensor_mul(y_hqb[:], py[0:D, :], rb[:])
                    nc.sync.dma_start(cc_in[h * D:(h + 1) * D, qsl], y_hqb[:])

        qkv_cm.__exit__(None, None, None)

        # ---- phase 3: exchange --------------------------------------
        import os as _os
        if _os.environ.get("KERNEL_NO_CC") == "1":
            # debug: skip the collective, copy local contribution only
            nc.gpsimd.dma_start(cc_out[0:HPC * D, :], cc_in[:])
            nc.gpsimd.dma_start(cc_out[HPC * D:2 * HPC * D, :], cc_in[:])
            nc.gpsimd.dma_start(cc_out[2 * HPC * D:3 * HPC * D, :], cc_in[:])
            nc.gpsimd.dma_start(cc_out[3 * HPC * D:4 * HPC * D, :], cc_in[:])
        else:
            nc.gpsimd.collective_compute(
                "AllGather",
                mybir.AluOpType.bypass,
                replica_groups=[[0, 1, 2, 3], [4, 5, 6, 7]],
                ins=[cc_in.opt()],
                outs=[cc_out.opt()],
            )

        # ---- phase 4: x2 = x + y, LN2, MLP --------------------------
        with tc.tile_pool(name="mlp", bufs=1) as mlp_pool, \
             tc.tile_pool(name="idx", bufs=1) as idx_pool, \
             tc.tile_pool(name="ln2_ps", bufs=1, space="PSUM") as ln2_ps, \
             tc.tile_pool(name="ln2_bc_ps", bufs=2, space="PSUM") as ln2_bc_ps, \
             tc.tile_pool(name="ln2_sb", bufs=1) as ln2_sb, \
             tc.tile_pool(name="sq2", bufs=3) as sq2_pool, \
             tc.tile_pool(name="wf", bufs=3) as wf_pool, \
             tc.tile_pool(name="wp", bufs=3) as wp_pool, \
             tc.tile_pool(name="m_ps", bufs=4, space="PSUM") as m_ps, \
             tc.tile_pool(name="o_sb", bufs=3) as o_sb:

            idx_sb = idx_pool.tile([128, CO], I32)
            nc.sync.dma_start(idx_sb[:], g_idx.rearrange("(o p) u -> p (o u)", p=128))
            import concourse.bass as bass_mod
            y_full = mlp_pool.tile([128, CO, TCH], F32R)
            cc_tbl = cc_out[:].rearrange("c (blk t) -> (c blk) t", t=TCH)
            for o in range(CO):
                nc.gpsimd.indirect_dma_start(
                    out=y_full[:, o, :],
                    out_offset=None,
                    in_=cc_tbl,
                    in_offset=bass_mod.IndirectOffsetOnAxis(
                        ap=idx_sb[:, o:o + 1], axis=0),
                )

            xres_sb = mlp_pool.tile([128, CO, TCH], F32R)
            nc.sync.dma_start(xres_sb[:],
                              x_res_t.rearrange("(o p) t -> p o t", p=128))
            x2 = y_full
            for o in range(CO):
                nc.vector.tensor_add(x2[:, o, :], xres_sb[:, o, :], y_full[:, o, :])

            # LN2 stats
            ps2_s = ln2_ps.tile([1, TCH], F32)
            ps2_q = ln2_ps.tile([1, TCH], F32)
            for o in range(CO):
                sq = sq2_pool.tile([128, TCH], F32R)
                nc.vector.tensor_mul(sq[:], x2[:, o, :], x2[:, o, :])
                nc.tensor.matmul(ps2_s[:], ones_c[:], x2[:, o, :],
                                 start=(o == 0), stop=(o == CO - 1))
                nc.tensor.matmul(ps2_q[:], ones_c[:], sq[:],
                                 start=(o == 0), stop=(o == CO - 1))
            mu2r = ln2_sb.tile([1, TCH], F32)
            msq2 = ln2_sb.tile([1, TCH], F32)
            var2 = ln2_sb.tile([1, TCH], F32)
            std2 = ln2_sb.tile([1, TCH], F32)
            rstd2 = ln2_sb.tile([1, TCH], F32R)
            nm2 = ln2_sb.tile([1, TCH], F32R)
            inv_c = 1.0 / C
            nc.vector.tensor_scalar_mul(mu2r[:], ps2_s[:], inv_c)
            nc.vector.tensor_scalar_mul(msq2[:], ps2_q[:], inv_c)
            nc.vector.tensor_mul(var2[:], mu2r[:], mu2r[:])
            nc.vector.tensor_tensor(var2[:], msq2[:], var2[:],
                                    mybir.AluOpType.subtract)
            nc.vector.tensor_scalar_add(var2[:], var2[:], EPS)
            nc.scalar.activation(std2[:], var2[:], AF.Sqrt)
            with nc.allow_low_precision(reason="f32r is 4-byte fp32"):
                nc.vector.reciprocal(rstd2[:], std2[:])
            nc.vector.tensor_mul(nm2[:], mu2r[:], rstd2[:])
            nc.vector.tensor_scalar_mul(nm2[:], nm2[:], -1.0)

            pb = ln2_bc_ps.tile([128, TCH], F32, tag="bc2")
            nc.tensor.matmul(pb[:], ones_r[:], rstd2[:], start=True, stop=True)
            rstd2_bc = ln2_sb.tile([128, TCH], F32R)
            nc.scalar.copy(rstd2_bc[:], pb[:])
            pb2 = ln2_bc_ps.tile([128, TCH], F32, tag="bc2")
            nc.tensor.matmul(pb2[:], ones_r[:], nm2[:], start=True, stop=True)
            nm2_bc = ln2_sb.tile([128, TCH], F32R)
            nc.scalar.copy(nm2_bc[:], pb2[:])

            h2 = mlp_pool.tile([128, CO, TCH], F32R)
            for o in range(CO):
                nc.vector.tensor_mul(h2[:, o, :], x2[:, o, :], rstd2_bc[:])
                nc.vector.tensor_add(h2[:, o, :], h2[:, o, :], nm2_bc[:])

            # fc + gelu
            bfc_sb = mlp_pool.tile([128, FO], F32)
            nc.sync.dma_start(bfc_sb[:], b_fc.rearrange("(o p) u -> p (o u)", p=128))
            m_sb = mlp_pool.tile([128, FO, TCH], F32R)
            wfc_r = w_fc.rearrange("(o p) m -> p o m", p=128)
            for mt in range(FO):
                wt = wf_pool.tile([128, CO, 128], F32R, tag="wfc")
                nc.sync.dma_start(wt[:], wfc_r[:, :, mt * 128:(mt + 1) * 128])
                pm = m_ps.tile([128, TCH], F32, tag="mm2")
                for o in range(CO):
                    nc.tensor.matmul(pm[:], wt[:, o, :], h2[:, o, :],
                                     start=(o == 0), stop=(o == CO - 1))
                nc.scalar.activation(m_sb[:, mt, :], pm[:], AF.Gelu,
                                     bias=bfc_sb[:, mt:mt + 1])

            # proj + bias + residual
            bpj_sb = mlp_pool.tile([128, CO], F32)
            nc.sync.dma_start(bpj_sb[:], b_proj.rearrange("(o p) u -> p (o u)", p=128))
            wpj_r = w_proj.rearrange("(o p) m -> p o m", p=128)
            out_r = out_t.rearrange("(o p) t -> p o t", p=128)
            for o in range(CO):
                wt_a = wp_pool.tile([128, FO // 2, 128], F32R, tag="wpj")
                wt_b = wp_pool.tile([128, FO // 2, 128], F32R, tag="wpj")
                nc.sync.dma_start(wt_a[:],
                                  wpj_r[:, 0:FO // 2, o * 128:(o + 1) * 128])
                nc.sync.dma_start(wt_b[:],
                                  wpj_r[:, FO // 2:FO, o * 128:(o + 1) * 128])
                pp = m_ps.tile([128, TCH], F32, tag="mm2")
                for kt in range(FO):
                    wt = wt_a[:, kt, :] if kt < FO // 2 else \
                        wt_b[:, kt - FO // 2, :]
                    nc.tensor.matmul(pp[:], wt, m_sb[:, kt, :],
                                     start=(kt == 0), stop=(kt == FO - 1))
                po_sb = o_sb.tile([128, TCH], F32, tag="po")
                nc.scalar.activation(po_sb[:], pp[:], AF.Identity,
                                     bias=bpj_sb[:, o:o + 1])
                fin = o_sb.tile([128, TCH], F32, tag="fin")
                nc.vector.tensor_add(fin[:], po_sb[:], x2[:, o, :])
                nc.sync.dma_start(out_r[:, o, :], fin[:])

        for cm in (dram_cm, const_cm):
            cm.__exit__(None, None, None)

    nc.compile()
    return nc


def _get_nc():
    if "nc" not in _CACHE:
        _CACHE["nc"] = _build()
    return _CACHE["nc"]


def _make_masks():
    m = np.zeros((4, 128, 512), np.float32)
    i = np.arange(128)[:, None]
    j = np.arange(512)[None, :]
    for t in range(4):
        m[t] = (128 * t + i <= j).astype(np.float32)
    return m


def kernel(x, ln1_g, ln1_b, W_attn, b_attn, ln2_g, ln2_b, W_fc, b_fc,
           W_proj, b_proj):
    global LAST_EXEC_NS, LAST_RESULTS
    import os

    from concourse.bass_utils import run_bass_kernel_spmd

    x = np.asarray(x, np.float32)
    W1 = np.asarray(ln1_g, np.float32)[:, None] * np.asarray(W_attn, np.float32)
    b1 = np.asarray(b_attn, np.float32) + np.asarray(ln1_b, np.float32) @ np.asarray(W_attn, np.float32)
    Wf = np.asarray(ln2_g, np.float32)[:, None] * np.asarray(W_fc, np.float32)
    bf = np.asarray(b_fc, np.float32) + np.asarray(ln2_b, np.float32) @ np.asarray(W_fc, np.float32)
    Wp = np.asarray(W_proj, np.float32)
    bp = np.asarray(b_proj, np.float32)

    masks = _make_masks()
    ones_col = np.ones((128, 1), np.float32)
    ones_row = np.ones((1, 128), np.float32)

    in_maps = []
    for c in range(N_CORES):
        b = c // G
        g = c % G
        tok0 = g * TCH
        qc = slice(g * HPC * D, (g + 1) * HPC * D)
        kc = slice(C + g * HPC * D, C + (g + 1) * HPC * D)
        vc = slice(2 * C + g * HPC * D, 2 * C + (g + 1) * HPC * D)
        xb_t = np.ascontiguousarray(x[b].T)
        in_maps.append({
            "x_t": xb_t,
            "x_res_t": np.ascontiguousarray(xb_t[:, tok0:tok0 + TCH]),
            "w_qk": np.ascontiguousarray(np.concatenate([W1[:, qc], W1[:, kc]], axis=1)),
            "b_qk": np.ascontiguousarray(np.concatenate([b1[qc], b1[kc]])[:, None]),
            "w_v": np.ascontiguousarray(W1[:, vc]),
            "b_v": np.ascontiguousarray(b1[vc][None, :]),
            "masks": masks,
            "ones_col": ones_col,
            "ones_row": ones_row,
            "w_fc": Wf,
            "b_fc": np.ascontiguousarray(bf[:, None]),
            "w_proj": Wp,
            "b_proj": np.ascontiguousarray(bp[:, None]),
            "g_idx": np.ascontiguousarray((G * np.arange(C) + g).astype(np.int32)[:, None]),
            "v_ones": np.ones((128, 16, 4, 1), np.float32),
        })

    nc = _get_nc()
    trace = os.environ.get("KERNEL_TRACE") == "1"
    kw = {}
    if trace:
        kw = dict(trace=True, trace_cores=list(range(N_CORES)))
    res = run_bass_kernel_spmd(nc, in_maps, core_ids=list(range(N_CORES)), **kw)
    LAST_EXEC_NS = res.exec_time_ns
    LAST_RESULTS = res

    out = np.empty((B, T, C), np.float32)
    for c in range(N_CORES):
        b = c // G
        tok0 = (c % G) * TCH
        out[b, tok0:tok0 + TCH, :] = res.results[c]["out_t"].T
    return out



# revision 30
# speedup vs baseline: 1.3307x; 1.3307x over previous
"""Trainium2 Bass kernel for a GPT-style transformer block (no attn out-proj).

Sharding (8 cores): attention is tensor-parallel over heads -- core c handles
batch c//4 and heads [4*(c%4), 4*(c%4)+4) over the full 2048-token causal
sequence. The MLP is token-parallel with a batch-interleaved assignment: core
r handles tokens [256*r, 256*r+256) of BOTH batches, which makes the
feature->token resharding a single 8-rank AllToAll (mesh) with no wasted
wire and no group-dependent addressing.

Attention softmax normalization is deferred past the collective: each core
ships unnormalized sums (plus the denominator row, via an appended ones
column in V) and the receiving core divides after the exchange.

Matmuls run in bf16 (fp32 PSUM accumulate). exp() is split between the
scalar engine (table exp) and the vector engine (Schraudolph bit-trick
exp, ~3% max rel err, which washes out in softmax normalization).
"""

import numpy as np

B, T, C = 2, 2048, 1024
H, D = 16, 64
HPC = 4            # heads per core
VD = HPC * D       # 256 v features per core
N_CORES = 8
TCH = 512          # tokens per core for the MLP (256 from each batch)
TB = 256           # per-batch token block
EPS = 1e-5
FC = 4 * C
CO = C // 128      # 8
FO = FC // 128     # 32
NT = T // 512      # 4 query chunks
TT = T // 128      # 16 token tiles
SLOT = HPC * (D + 1)   # 260 rows per A2A shard slot

# Schraudolph exp: exp(0.125*s) ~= bitcast(int32(A*s + BEXP))
A_EXP = 0.125 * 1.4426950408889634 * (1 << 23)
B_EXP = (127.0 - 0.0430357) * (1 << 23)

_CACHE = {}
LAST_EXEC_NS = None
LAST_RESULTS = None


def _build():
    import concourse.tile as tile
    from concourse import bacc, mybir

    F32 = mybir.dt.float32
    F32R = mybir.dt.float32r
    BF16 = mybir.dt.bfloat16
    I32 = mybir.dt.int32
    AF = mybir.ActivationFunctionType
    ADD = mybir.AluOpType.add
    SUB = mybir.AluOpType.subtract
    MUL = mybir.AluOpType.mult

    nc = bacc.Bacc("TRN2", target_bir_lowering=False, debug=False,
                   num_devices=N_CORES)

    def inp(name, shape, dt):
        return nc.dram_tensor(name, shape, dt, kind="ExternalInput").ap()

    x_t = inp("x_t", [C, T], BF16)
    x_res = inp("x_res", [C, TCH], F32)
    w_qk = inp("w_qk", [128, CO, 512], BF16)
    b_qk = inp("b_qk", [128, 4], F32)
    w_v = inp("w_v", [128, CO, VD], BF16)
    b_v = inp("b_v", [1, VD], BF16)
    masks = inp("masks", [4, 128, 512], F32)
    w_fc = inp("w_fc", [128, CO, FC], BF16)
    b_fc = inp("b_fc", [128, FO], F32)
    w_pj = inp("w_pj", [C, FC], BF16)      # row o*128+p, col kt*128+m
    b_pj = inp("b_pj", [128, CO], F32)
    sel = inp("sel", [H, C], BF16)
    ones_col = inp("ones_col", [128, 1], BF16)
    ones_row = inp("ones_row", [1, 128], BF16)
    v_ones = inp("v_ones", [128, TT, HPC, 1], BF16)

    out_t = nc.dram_tensor("out_t", [C, TCH], F32, kind="ExternalOutput").ap()

    with tile.TileContext(nc) as tc:
        # ---- persistent pools (enter order = reverse close order) ----
        const_cm = tc.tile_pool(name="const", bufs=1)
        dram_cm = tc.tile_pool(name="dram", bufs=1, space="DRAM")
        wmlp_cm = tc.tile_pool(name="wmlp", bufs=1)
        qkv_cm = tc.tile_pool(name="qkv", bufs=1)
        const = const_cm.__enter__()
        dram = dram_cm.__enter__()
        wmlp = wmlp_cm.__enter__()
        qkv_pool = qkv_cm.__enter__()

        ones_cb = const.tile([128, 1], BF16)      # stats reduce stationary
        ones_rb = const.tile([1, 128], BF16)      # broadcast stationary
        nc.sync.dma_start(ones_cb[:], ones_col)
        nc.sync.dma_start(ones_rb[:], ones_row)
        mask_sb = const.tile([128, 4, 512], F32)
        sel_sb = const.tile([H, C], BF16)
        bqk_sb = const.tile([128, 4], F32)
        bfc_sb = const.tile([128, FO], F32)
        bpj_sb = const.tile([128, CO], F32)
        nc.sync.dma_start(mask_sb[:], masks.rearrange("m p f -> p m f"))
        nc.sync.dma_start(sel_sb[:], sel)
        nc.sync.dma_start(bqk_sb[:], b_qk)
        nc.sync.dma_start(bfc_sb[:], b_fc)
        nc.sync.dma_start(bpj_sb[:], b_pj)

        wfc_sb = wmlp.tile([128, CO, FC], BF16)     # 8 MB, prefetched
        x2 = wmlp.tile([128, CO, TCH], F32)         # x_res, then x2

        q_sb = qkv_pool.tile([128, 2, T], BF16)
        k_sb = qkv_pool.tile([128, 2, T], BF16)
        v_sb = qkv_pool.tile([128, TT, SLOT], BF16)   # [.,16,260]
        v4 = v_sb[:].rearrange("p t (h e) -> p t h e", h=HPC)
        nc.sync.dma_start(v4[:, :, :, D:D + 1], v_ones)

        cc_in = dram.tile([N_CORES * SLOT, TB], BF16)    # [2080, 256]
        cc_out = dram.tile([N_CORES * SLOT, TB], BF16)

        # ---- phase 1: load x, LN1, QKV -------------------------------
        with tc.tile_pool(name="xh", bufs=1) as xh_pool, \
             tc.tile_pool(name="ln_ps", bufs=1, space="PSUM") as ln_ps, \
             tc.tile_pool(name="bc_ps", bufs=2, space="PSUM") as bc_ps, \
             tc.tile_pool(name="rows", bufs=2) as rows, \
             tc.tile_pool(name="sq", bufs=3) as sq_pool, \
             tc.tile_pool(name="mm_ps", bufs=3, space="PSUM") as mm_ps:

            xh = xh_pool.tile([128, CO, T], BF16)
            wqk_sb = xh_pool.tile([128, CO, 512], BF16)
            wv_sb = xh_pool.tile([128, CO, VD], BF16)
            bv_row = xh_pool.tile([1, VD], BF16)
            nc.sync.dma_start(xh[:], x_t.rearrange("(o p) t -> p o t", p=128))
            nc.sync.dma_start(wqk_sb[:], w_qk)
            nc.sync.dma_start(wv_sb[:], w_v)
            nc.sync.dma_start(bv_row[:], b_v)
            # bulk prefetches for later phases (separate trigger queues)
            nc.gpsimd.dma_start(wfc_sb[:], w_fc)
            nc.gpsimd.dma_start(x2[:], x_res.rearrange("(o p) t -> p o t",
                                                       p=128))

            # b_v broadcast to [128, VD]
            pbv = bc_ps.tile([128, 512], F32, tag="bc")
            nc.tensor.matmul(pbv[:, 0:VD], ones_rb[:],
                             bv_row[:], start=True, stop=True)
            bvbc_sb = xh_pool.tile([128, VD], BF16)
            nc.vector.tensor_copy(bvbc_sb[:], pbv[:, 0:VD])

            inv_c = 1.0 / C
            for cn in range(NT):
                sl = slice(cn * 512, cn * 512 + 512)
                ps_s = ln_ps.tile([1, 512], F32, tag="ps_s")
                ps_q = ln_ps.tile([1, 512], F32, tag="ps_q")
                for o in range(CO):
                    sq = sq_pool.tile([128, 512], BF16)
                    nc.vector.tensor_mul(sq[:], xh[:, o, sl], xh[:, o, sl])
                    nc.tensor.matmul(ps_s[:], ones_cb[:], xh[:, o, sl],
                                     start=(o == 0), stop=(o == CO - 1))
                    nc.tensor.matmul(ps_q[:], ones_cb[:], sq[:],
                                     start=(o == 0), stop=(o == CO - 1))
                mu = rows.tile([1, 512], F32, tag="mu")
                msq = rows.tile([1, 512], F32, tag="msq")
                var = rows.tile([1, 512], F32, tag="var")
                std = rows.tile([1, 512], F32, tag="std")
                rstd = rows.tile([1, 512], F32, tag="rstd")
                nc.vector.tensor_scalar_mul(mu[:], ps_s[:], inv_c)
                nc.vector.tensor_scalar_mul(msq[:], ps_q[:], inv_c)
                nc.vector.tensor_mul(var[:], mu[:], mu[:])
                nc.vector.tensor_tensor(var[:], msq[:], var[:], SUB)
                nc.vector.tensor_scalar_add(var[:], var[:], EPS)
                nc.scalar.activation(std[:], var[:], AF.Sqrt)
                nc.vector.reciprocal_approx_fast(rstd[:], std[:])
                rstd_bf = rows.tile([1, 512], BF16, tag="rstd_bf")
                nm_bf = rows.tile([1, 512], BF16, tag="nm_bf")
                nc.vector.tensor_copy(rstd_bf[:], rstd[:])
                nc.vector.scalar_tensor_tensor(nm_bf[:], mu[:], -1.0, rstd[:],
                                               MUL, MUL)

                pb = bc_ps.tile([128, 512], F32, tag="bc")
                nc.tensor.matmul(pb[:], ones_rb[:], rstd_bf[:],
                                 start=True, stop=True)
                rstd_bc = rows.tile([128, 512], BF16, tag="rstd_bc")
                nc.vector.tensor_copy(rstd_bc[:], pb[:])
                pb2 = bc_ps.tile([128, 512], F32, tag="bc")
                nc.tensor.matmul(pb2[:], ones_rb[:], nm_bf[:],
                                 start=True, stop=True)
                nm_bc = rows.tile([128, 512], BF16, tag="nm_bc")
                nc.vector.tensor_copy(nm_bc[:], pb2[:])

                for o in range(CO):
                    nc.vector.tensor_mul(xh[:, o, sl], xh[:, o, sl],
                                         rstd_bc[:])
                    nc.vector.tensor_add(xh[:, o, sl], xh[:, o, sl], nm_bc[:])

                # q, k for this chunk
                for m in range(4):
                    pq = mm_ps.tile([128, 512], F32, tag="mm")
                    for o in range(CO):
                        nc.tensor.matmul(pq[:],
                                         wqk_sb[:, o, m * 128:(m + 1) * 128],
                                         xh[:, o, sl],
                                         start=(o == 0), stop=(o == CO - 1))
                    dest = q_sb[:, m, sl] if m < 2 else k_sb[:, m - 2, sl]
                    nc.scalar.activation(dest, pq[:], AF.Identity,
                                         bias=bqk_sb[:, m:m + 1])

                # v (token-major) for the 4 token tiles of this chunk
                for tt in range(4 * cn, 4 * cn + 4):
                    tsl = slice(tt * 128, tt * 128 + 128)
                    pv_full = mm_ps.tile([128, 512], F32, tag="mm", name="pv")
                    pv = pv_full[:, 0:VD]
                    for o in range(CO):
                        nc.tensor.matmul(pv[:], xh[:, o, tsl], wv_sb[:, o, :],
                                         start=(o == 0), stop=(o == CO - 1))
                    vview = v_sb[:, tt, :].rearrange("p (h e) -> p h e",
                                                     e=D + 1)
                    nc.vector.tensor_tensor(
                        vview[:, :, 0:D],
                        pv[:].rearrange("p (h e) -> p h e", e=D),
                        bvbc_sb[:].rearrange("p (h e) -> p h e", e=D), ADD)

        # ---- phase 2: attention --------------------------------------
        with tc.tile_pool(name="a", bufs=2) as a_pool, \
             tc.tile_pool(name="iexp", bufs=3) as i_pool, \
             tc.tile_pool(name="s_ps", bufs=4, space="PSUM") as s_ps, \
             tc.tile_pool(name="y_ps", bufs=2, space="PSUM") as y_ps, \
             tc.tile_pool(name="stage", bufs=3) as stage_pool:

            for h in range(HPC):
                po = 64 * (h % 2)
                pt = h // 2
                for qb in range(NT):
                    qsl = slice(qb * 512, qb * 512 + 512)
                    nkv = 4 * qb + 4
                    a_t = a_pool.tile([128, TT, 512], BF16, tag="a")
                    for t in range(nkv):
                        ps = s_ps.tile([128, 512], F32)
                        nc.tensor.matmul(
                            ps[:],
                            k_sb[po:po + 64, pt, t * 128:(t + 1) * 128],
                            q_sb[po:po + 64, pt, qsl],
                            start=True, stop=True)
                        diag = t >= 4 * qb
                        if diag or (t % 4 == 3):
                            # Schraudolph exp on DVE
                            it = i_pool.tile([128, 512], I32)
                            nc.vector.tensor_scalar(
                                it[:], ps[:], A_EXP, B_EXP, MUL, ADD)
                            if diag:
                                nc.vector.tensor_mul(
                                    a_t[:, t, :], it[:].bitcast(F32),
                                    mask_sb[:, t - 4 * qb, :])
                            else:
                                nc.vector.tensor_copy(a_t[:, t, :],
                                                      it[:].bitcast(F32))
                        else:
                            nc.scalar.activation(a_t[:, t, :], ps[:], AF.Exp,
                                                 scale=0.125)
                    py = y_ps.tile([D + 1, 512], F32)
                    for t in range(nkv):
                        nc.tensor.matmul(
                            py[:],
                            v_sb[:, t, h * (D + 1):(h + 1) * (D + 1)],
                            a_t[:, t, :],
                            start=(t == 0), stop=(t == nkv - 1))
                    stg = stage_pool.tile([D + 1, 512], BF16, tag="stg")
                    nc.vector.tensor_copy(stg[:], py[:])
                    # scatter [65, 512] -> slots (2qb, 2qb+1), rows 65h..
                    dst = cc_in[:].rearrange("(j r) t -> r j t", j=N_CORES)[
                        65 * h:65 * h + 65, 2 * qb:2 * qb + 2, :]
                    nc.sync.dma_start(
                        dst, stg[:].rearrange("r (s t) -> r s t", s=2))

        qkv_cm.__exit__(None, None, None)

        # ---- phase 3: exchange (8-rank AllToAll, mesh) ---------------
        nc.gpsimd.collective_compute(
            "AllToAll",
            mybir.AluOpType.bypass,
            replica_groups=[list(range(N_CORES))],
            ins=[cc_in.opt()],
            outs=[cc_out.opt()],
        )

        # ---- phase 4: y assemble, x2, LN2, MLP -----------------------
        with tc.tile_pool(name="mlp", bufs=1) as mlp_pool, \
             tc.tile_pool(name="ln2_ps", bufs=1, space="PSUM") as ln2_ps, \
             tc.tile_pool(name="bc2_ps", bufs=2, space="PSUM") as bc2_ps, \
             tc.tile_pool(name="rows2", bufs=1) as rows2, \
             tc.tile_pool(name="sq2", bufs=3) as sq2_pool, \
             tc.tile_pool(name="wp", bufs=3) as wp_pool, \
             tc.tile_pool(name="m_ps", bufs=3, space="PSUM") as m_ps, \
             tc.tile_pool(name="o_sb", bufs=3) as o_sb:

            y_sb = mlp_pool.tile([128, CO, TCH], BF16)
            den_bf = mlp_pool.tile([H, TCH], BF16)
            src_all = cc_out[:].rearrange("(bb g l r) t -> g l r bb t",
                                          bb=2, g=4, l=HPC, r=D + 1)
            for o in range(CO):
                g = o // 2
                l0 = 2 * (o % 2)
                for q in range(2):
                    src = src_all[g, l0 + q, 0:D, :, :]       # [64,2,256]
                    dst = y_sb[64 * q:64 * q + 64, o, :].rearrange(
                        "d (bb t) -> d bb t", bb=2)
                    nc.sync.dma_start(dst, src)
            den_src = cc_out[:].rearrange(
                "(bb hh r) t -> hh r bb t", bb=2, hh=H, r=D + 1)[
                :, D:D + 1, :, :]
            nc.sync.dma_start(
                den_bf[:].rearrange("hh (u bb t) -> hh u bb t", u=1, bb=2),
                den_src)

            den_f = rows2.tile([H, TCH], F32)
            rr_f = rows2.tile([H, TCH], F32)
            rr_bf = rows2.tile([H, TCH], BF16)
            nc.vector.tensor_copy(den_f[:], den_bf[:])
            nc.vector.reciprocal_approx_fast(rr_f[:], den_f[:])
            nc.vector.tensor_copy(rr_bf[:], rr_f[:])

            x2bf = mlp_pool.tile([128, CO, TCH], BF16)
            for o in range(CO):
                prr = bc2_ps.tile([128, TCH], F32, tag="bc2")
                nc.tensor.matmul(prr[:], sel_sb[:, o * 128:(o + 1) * 128],
                                 rr_bf[:], start=True, stop=True)
                rrbc = o_sb.tile([128, TCH], BF16, tag="rrbc")
                nc.vector.tensor_copy(rrbc[:], prr[:])
                yn = o_sb.tile([128, TCH], F32, tag="yn")
                nc.vector.tensor_mul(yn[:], y_sb[:, o, :], rrbc[:])
                nc.vector.tensor_add(x2[:, o, :], x2[:, o, :], yn[:])
                nc.vector.tensor_copy(x2bf[:, o, :], x2[:, o, :])

            # LN2 over the 512 on-core tokens
            ps2_s = ln2_ps.tile([1, TCH], F32, tag="s")
            ps2_q = ln2_ps.tile([1, TCH], F32, tag="q")
            for o in range(CO):
                sq = sq2_pool.tile([128, TCH], BF16)
                nc.vector.tensor_mul(sq[:], x2bf[:, o, :], x2bf[:, o, :])
                nc.tensor.matmul(ps2_s[:], ones_cb[:], x2bf[:, o, :],
                                 start=(o == 0), stop=(o == CO - 1))
                nc.tensor.matmul(ps2_q[:], ones_cb[:], sq[:],
                                 start=(o == 0), stop=(o == CO - 1))
            mu2 = rows2.tile([1, TCH], F32)
            msq2 = rows2.tile([1, TCH], F32)
            var2 = rows2.tile([1, TCH], F32)
            std2 = rows2.tile([1, TCH], F32)
            rstd2 = rows2.tile([1, TCH], F32)
            inv_c = 1.0 / C
            nc.vector.tensor_scalar_mul(mu2[:], ps2_s[:], inv_c)
            nc.vector.tensor_scalar_mul(msq2[:], ps2_q[:], inv_c)
            nc.vector.tensor_mul(var2[:], mu2[:], mu2[:])
            nc.vector.tensor_tensor(var2[:], msq2[:], var2[:], SUB)
            nc.vector.tensor_scalar_add(var2[:], var2[:], EPS)
            nc.scalar.activation(std2[:], var2[:], AF.Sqrt)
            nc.vector.reciprocal_approx_fast(rstd2[:], std2[:])
            rstd2_bf = rows2.tile([1, TCH], BF16)
            nm2_bf = rows2.tile([1, TCH], BF16)
            nc.vector.tensor_copy(rstd2_bf[:], rstd2[:])
            nc.vector.scalar_tensor_tensor(nm2_bf[:], mu2[:], -1.0, rstd2[:],
                                           MUL, MUL)

            pb = bc2_ps.tile([128, TCH], F32, tag="bc2")
            nc.tensor.matmul(pb[:], ones_rb[:], rstd2_bf[:],
                             start=True, stop=True)
            rstd2_bc = rows2.tile([128, TCH], BF16)
            nc.vector.tensor_copy(rstd2_bc[:], pb[:])
            pb2 = bc2_ps.tile([128, TCH], F32, tag="bc2")
            nc.tensor.matmul(pb2[:], ones_rb[:], nm2_bf[:],
                             start=True, stop=True)
            nm2_bc = rows2.tile([128, TCH], BF16)
            nc.vector.tensor_copy(nm2_bc[:], pb2[:])

            h2 = x2bf   # normalize in place
            for o in range(CO):
                nc.vector.tensor_mul(h2[:, o, :], h2[:, o, :], rstd2_bc[:])
                nc.vector.tensor_add(h2[:, o, :], h2[:, o, :], nm2_bc[:])

            # fc + gelu (weights resident in SBUF)
            m_sb = mlp_pool.tile([128, FO, TCH], BF16)
            for mt in range(FO):
                pm = m_ps.tile([128, TCH], F32, tag="mm2")
                for o in range(CO):
                    nc.tensor.matmul(pm[:],
                                     wfc_sb[:, o, mt * 128:(mt + 1) * 128],
                                     h2[:, o, :],
                                     start=(o == 0), stop=(o == CO - 1))
                nc.scalar.activation(m_sb[:, mt, :], pm[:], AF.Gelu,
                                     bias=bfc_sb[:, mt:mt + 1])

            # proj + bias + residual (weights streamed per o-tile)
            out_r = out_t.rearrange("(o p) t -> p o t", p=128)
            wpj_r = w_pj.rearrange("(o p) (k m) -> o p k m", p=128, m=128)
            for o in range(CO):
                wt = wp_pool.tile([128, FO, 128], BF16, tag="wpj")
                nc.sync.dma_start(wt[:], wpj_r[o])
                pp = m_ps.tile([128, TCH], F32, tag="mm2")
                for kt in range(FO):
                    nc.tensor.matmul(pp[:], wt[:, kt, :], m_sb[:, kt, :],
                                     start=(kt == 0), stop=(kt == FO - 1))
                po_sb = o_sb.tile([128, TCH], F32, tag="po")
                nc.scalar.activation(po_sb[:], pp[:], AF.Identity,
                                     bias=bpj_sb[:, o:o + 1])
                fin = o_sb.tile([128, TCH], F32, tag="fin")
                nc.vector.tensor_add(fin[:], po_sb[:], x2[:, o, :])
                nc.sync.dma_start(out_r[:, o, :], fin[:])

        for cm in (wmlp_cm, dram_cm, const_cm):
            cm.__exit__(None, None, None)

    nc.compile()
    return nc


def _get_nc():
    if "nc" not in _CACHE:
        _CACHE["nc"] = _build()
    return _CACHE["nc"]


def _make_masks():
    m = np.zeros((4, 128, 512), np.float32)
    i = np.arange(128)[:, None]
    j = np.arange(512)[None, :]
    for t in range(4):
        m[t] = (128 * t + i <= j).astype(np.float32)
    return m


def kernel(x, ln1_g, ln1_b, W_attn, b_attn, ln2_g, ln2_b, W_fc, b_fc,
           W_proj, b_proj):
    global LAST_EXEC_NS, LAST_RESULTS
    import os
    import ml_dtypes

    from concourse.bass_utils import run_bass_kernel_spmd

    BF = ml_dtypes.bfloat16

    x = np.asarray(x, np.float32)
    W1 = np.asarray(ln1_g, np.float32)[:, None] * np.asarray(W_attn, np.float32)
    b1 = np.asarray(b_attn, np.float32) + \
        np.asarray(ln1_b, np.float32) @ np.asarray(W_attn, np.float32)
    Wf = np.asarray(ln2_g, np.float32)[:, None] * np.asarray(W_fc, np.float32)
    bf = np.asarray(b_fc, np.float32) + \
        np.asarray(ln2_b, np.float32) @ np.asarray(W_fc, np.float32)
    Wp = np.asarray(W_proj, np.float32)
    bp = np.asarray(b_proj, np.float32)

    masks = _make_masks()

    wfc_l = np.ascontiguousarray(
        Wf.reshape(CO, 128, FC).transpose(1, 0, 2)).astype(BF)
    # w_pj[o*128+p, kt*128+m] = Wp[kt*128+p ??? see build: stationary
    # wt[p, kt, m] must equal Wp[kt*128+p, o*128+m]
    wpj_l = np.ascontiguousarray(
        Wp.reshape(FO, 128, CO, 128).transpose(2, 1, 0, 3).reshape(C, FC)
    ).astype(BF)
    bfc_l = np.ascontiguousarray(bf.reshape(FO, 128).T)
    bpj_l = np.ascontiguousarray(bp.reshape(CO, 128).T)

    # sel one-hot: rr_bc[p, t] = rr[2*o + p//64, t]
    sel = np.zeros((H, C), np.float32)
    for o in range(CO):
        for p in range(128):
            sel[2 * o + p // 64, o * 128 + p] = 1.0
    sel = sel.astype(BF)

    xT = [np.ascontiguousarray(x[b].T) for b in range(B)]

    in_maps = []
    for c in range(N_CORES):
        b = c // 4
        g = c % 4
        qc = slice(g * HPC * D, (g + 1) * HPC * D)
        kc = slice(C + g * HPC * D, C + (g + 1) * HPC * D)
        vc = slice(2 * C + g * HPC * D, 2 * C + (g + 1) * HPC * D)
        wqk = np.concatenate([W1[:, qc], W1[:, kc]], axis=1)      # [1024,512]
        wv = W1[:, vc]                                            # [1024,256]
        tok0 = TB * c
        xres = np.concatenate(
            [xT[0][:, tok0:tok0 + TB], xT[1][:, tok0:tok0 + TB]], axis=1)
        in_maps.append({
            "x_t": xT[b].astype(BF),
            "x_res": np.ascontiguousarray(xres),
            "w_qk": np.ascontiguousarray(
                wqk.reshape(CO, 128, 512).transpose(1, 0, 2)).astype(BF),
            "b_qk": np.ascontiguousarray(
                np.concatenate([b1[qc], b1[kc]]).reshape(4, 128).T),
            "w_v": np.ascontiguousarray(
                wv.reshape(CO, 128, VD).transpose(1, 0, 2)).astype(BF),
            "b_v": np.ascontiguousarray(b1[vc][None, :]).astype(BF),
            "masks": masks,
            "w_fc": wfc_l,
            "b_fc": bfc_l,
            "w_pj": wpj_l,
            "b_pj": bpj_l,
            "sel": sel,
            "ones_col": np.ones((128, 1), np.float32).astype(BF),
            "ones_row": np.ones((1, 128), np.float32).astype(BF),
            "v_ones": np.ones((128, TT, HPC, 1), np.float32).astype(BF),
        })

    nc = _get_nc()
    trace = os.environ.get("KERNEL_TRACE") == "1"
    kw = {}
    if trace:
        kw = dict(trace=True, trace_cores=list(range(N_CORES)))
    res = run_bass_kernel_spmd(nc, in_maps, core_ids=list(range(N_CORES)), **kw)
    LAST_EXEC_NS = res.exec_time_ns
    LAST_RESULTS = res

    out = np.empty((B, T, C), np.float32)
    for c in range(N_CORES):
        tok0 = TB * c
        r = res.results[c]["out_t"]
        out[0, tok0:tok0 + TB, :] = r[:, 0:TB].T
        out[1, tok0:tok0 + TB, :] = r[:, TB:2 * TB].T
    return out
